# revision 15
# baseline (speedup 1.0000x reference)
"""Trainium2 Bass kernel for nn_BiEvidenceNet.

Model (B=1024, R=512, D=256):
    width  = clip(exp(log_width), 1e-3, 50)                  (R,D)
    t_low  = center - width/2 ; t_high = center + width/2    (R,D)
    kappa  = clip(exp(log_kappa), 0.5, 50)                   scalar
    low    = sigmoid(kappa*(t_low - x))   high = sigmoid(kappa*(x - t_high))
    evidence[b,r] = sum_d m*(el*(2*low-1) + eh*(2*high-1))   m=sig(mask), el/eh=tanh(e_*)
    z = sigmoid(6*(evidence - t));  y = z @ head_w.T + head_b

Key identity: 2*sigmoid(u)-1 = tanh(u/2). When t_low / t_high are constant
across the rule axis (true at init: center == 0, log_width == 0 -- verified at
runtime), the (B,R,D) broadcast collapses to two matmuls:
    T_low[b,d]  = tanh(kappa/2*(tau_low[d]  - x[b,d]))
    T_high[b,d] = tanh(kappa/2*(x[b,d] - tau_high[d]))
    evidence    = T_low @ (m*el).T + T_high @ (m*eh).T
Sharding: data-parallel over B, 128 rows/core; (R,D) params replicated.
On-core layout keeps D on partitions (2 k-tiles of 128) so both matmul
operands are naturally transposed; evidence lands as (128 b, 512 r) in PSUM,
-t enters as a rank-1 matmul update, and the head is a fused DVE
multiply+reduce over the free (rule) axis.
"""

import numpy as np

B, R, D = 1024, 512, 256
N_CORES = 8
BS = B // N_CORES          # batch rows per core
KT = D // 128              # contraction k-tiles
BETA = 6.0

_F32 = np.float32


def _single_wait_tile_context(nc, tile):
    """TileContext whose tail drain carries at most one sync wait.

    This toolchain's walrus encodes at most ONE sync wait per instruction
    ("Too many sync wait commands" otherwise).  Tile's stock kernel-tail
    drain waits on every engine/DMA proc at once, so split those waits
    across single-wait NOPs on the sync engine first.
    """
    from concourse.vector_clock import ScopedClock, VectorClock

    class SingleWaitTileContext(tile.TileContext):
        def _drain_and_barrier(self, tick_clock, wait_clock):
            gc = tick_clock.global_clock
            n = len(gc)
            for proc in range(n):
                if gc[proc] <= 0:
                    continue
                vec = VectorClock([gc[i] if i == proc else 0 for i in range(n)])
                inst = self.nc.sync.nop(nofuse=True)
                wait_clock.add_sem_waits(inst.ins, ScopedClock({None: vec}))
            # the NOP chain above already waited out every proc, so the drain
            # itself needs no waits (walrus would reject a multi-wait drain)
            self.nc.sync.drain()
            self.nc.all_engine_barrier()
            assert self.sems is not None
            popped = self.nc._tile_sem_poison_stack.pop()
            assert popped is self._sem_poison
            self.nc.clear_and_free_semaphores(list(self.sems.allocated().values()))
            self.nc.all_engine_barrier()

    return SingleWaitTileContext(nc)


def _build_nc(scale_lo: float, scale_hi: float, head_b: float):
    import concourse.bass as bass
    import concourse.mybir as mybir
    from concourse import tile

    f32 = mybir.dt.float32
    AF = mybir.ActivationFunctionType
    ALU = mybir.AluOpType

    nc = bass.Bass()
    # xb packs the x shard (transposed) with the two per-partition activation
    # bias columns so each T activation depends on exactly one DMA semaphore
    # (walrus rejects biased ACT instructions carrying >1 sync wait).
    d_xb = nc.declare_dram_parameter("xb", [KT, 128, BS + 2], f32, isOutput=False)
    d_maskT = nc.declare_dram_parameter("maskT", [KT, 128, R], f32, isOutput=False)
    d_elT = nc.declare_dram_parameter("elT", [KT, 128, R], f32, isOutput=False)
    d_ehT = nc.declare_dram_parameter("ehT", [KT, 128, R], f32, isOutput=False)
    d_t = nc.declare_dram_parameter("t_row", [1, R], f32, isOutput=False)
    d_w = nc.declare_dram_parameter("w_bcast", [BS, R], f32, isOutput=False)
    d_y = nc.declare_dram_parameter("y", [BS, 1], f32, isOutput=True)

    with _single_wait_tile_context(nc, tile) as tc:
        with (
            tc.tile_pool(name="sb", bufs=1) as sb,
            tc.tile_pool(name="ps", bufs=1, space="PSUM") as ps,
        ):
            xt = sb.tile([128, KT, BS + 2], f32, tag="xt")
            mkt = sb.tile([128, KT, R], f32, tag="mkt")
            elt = sb.tile([128, KT, R], f32, tag="elt")
            eht = sb.tile([128, KT, R], f32, tag="eht")
            tr = sb.tile([1, R], f32, tag="tr")
            wb = sb.tile([BS, R], f32, tag="wb")

            # one DMA per logical tensor (k-tiles folded into the free dim via
            # a strided source AP) so each consumer waits on a single DMA sem
            nc.sync.dma_start(xt[:], d_xb[:].rearrange("k p b -> p k b"))
            nc.sync.dma_start(tr[:], d_t[:])
            nc.sync.dma_start(mkt[:], d_maskT[:].rearrange("k p r -> p k r"))
            nc.sync.dma_start(elt[:], d_elT[:].rearrange("k p r -> p k r"))
            nc.sync.dma_start(eht[:], d_ehT[:].rearrange("k p r -> p k r"))
            nc.sync.dma_start(wb[:], d_w[:])

            negt = sb.tile([1, R], f32, tag="negt")
            nc.vector.tensor_scalar_mul(negt[:], tr[:], -1.0)

            # T tiles, kept transposed: partition = d, free = b
            tlo = sb.tile([128, KT, BS], f32, tag="tlo")
            thi = sb.tile([128, KT, BS], f32, tag="thi")
            for k in range(KT):
                nc.scalar.activation(
                    tlo[:, k, :], xt[:, k, 0:BS], AF.Tanh,
                    bias=xt[:, k, BS:BS + 1], scale=scale_lo,
                )
                nc.scalar.activation(
                    thi[:, k, :], xt[:, k, 0:BS], AF.Tanh,
                    bias=xt[:, k, BS + 1:BS + 2], scale=scale_hi,
                )

            # rule-param prep: A = sigmoid(mask)*tanh(e_low), B2 = ... e_high
            m = sb.tile([128, KT, R], f32, tag="m")
            el = sb.tile([128, KT, R], f32, tag="el")
            eh = sb.tile([128, KT, R], f32, tag="eh")
            nc.scalar.activation(m[:], mkt[:], AF.Sigmoid)
            nc.scalar.activation(el[:], elt[:], AF.Tanh)
            nc.scalar.activation(eh[:], eht[:], AF.Tanh)
            a_t = sb.tile([128, KT, R], f32, tag="a_t")
            b_t = sb.tile([128, KT, R], f32, tag="b_t")
            nc.vector.tensor_mul(a_t[:], m[:], el[:])
            nc.vector.tensor_mul(b_t[:], m[:], eh[:])

            # The ISA fits only one sync-wait on LDW/ACT/TTR instruction
            # structs, so every op below must depend on a single semaphore.
            # `wcheck` pulls the w_bcast DMA tick onto DVE (so the TTR later
            # needs only the ACT wait); `ones` reads b_t so its DVE tick
            # transitively covers a_t/b_t for the leading rank-1 matmul.
            wcheck = sb.tile([1, 1], f32, tag="wcheck")
            nc.vector.tensor_scalar_mul(wcheck[:], wb[0:1, 0:1], 1.0)
            ones = sb.tile([1, BS], f32, tag="ones")
            nc.vector.tensor_scalar(
                ones[:], b_t[0:1, 0, 0:BS], 0.0, 1.0, ALU.mult, ALU.add)

            # evidence - t, accumulated in one PSUM bank: (128 b, 512 r).
            # rank-1 (-t) term first: its DVE wait covers a_t/b_t, so the
            # four big matmuls each carry exactly one ACT wait.
            ev = ps.tile([128, R], f32, tag="ev")
            nc.tensor.matmul(ev[:], ones[:], negt[:], start=True, stop=False)
            nc.tensor.matmul(ev[:], tlo[:, 0, :], a_t[:, 0, :], start=False, stop=False)
            nc.tensor.matmul(ev[:], thi[:, 0, :], b_t[:, 0, :], start=False, stop=False)
            nc.tensor.matmul(ev[:], tlo[:, 1, :], a_t[:, 1, :], start=False, stop=False)
            nc.tensor.matmul(ev[:], thi[:, 1, :], b_t[:, 1, :], start=False, stop=True)

            z = sb.tile([128, R], f32, tag="z")
            nc.scalar.activation(z[:], ev[:], AF.Sigmoid, scale=BETA)

            # head: y = sum_r z*w + head_b (TensorTensorReduce miscompiles on
            # this walrus build — "ISA wrong length" — so spell it out)
            zw = sb.tile([128, R], f32, tag="zw")
            yt = sb.tile([128, 1], f32, tag="yt")
            nc.vector.tensor_mul(zw[:], z[:], wb[:])
            nc.vector.tensor_reduce(
                yt[:], zw[:], axis=mybir.AxisListType.X, op=ALU.add)
            nc.vector.tensor_scalar_add(yt[:], yt[:], head_b)
            nc.sync.dma_start(d_y[:], yt[:])

    nc.finalize()
    return nc


def _fast_path_inputs(x, mask, e_low, e_high, tau_lo, tau_hi, kappa, t, head_w):
    """Build the per-core input maps (host work = transposes/slicing only)."""
    khalf = _F32(kappa) / _F32(2.0)
    blo = (khalf * tau_lo).astype(_F32).reshape(KT, 128)
    bhi = (-khalf * tau_hi).astype(_F32).reshape(KT, 128)
    maskT = np.ascontiguousarray(mask.T.reshape(KT, 128, R), dtype=_F32)
    elT = np.ascontiguousarray(e_low.T.reshape(KT, 128, R), dtype=_F32)
    ehT = np.ascontiguousarray(e_high.T.reshape(KT, 128, R), dtype=_F32)
    t_row = np.ascontiguousarray(t.reshape(1, R), dtype=_F32)
    w_b = np.ascontiguousarray(np.broadcast_to(head_w.reshape(1, R), (BS, R)), dtype=_F32)
    xT = np.ascontiguousarray(x.T, dtype=_F32)  # (D, B)

    in_maps = []
    for i in range(N_CORES):
        xb = np.empty((KT, 128, BS + 2), dtype=_F32)
        xb[:, :, :BS] = xT[:, i * BS:(i + 1) * BS].reshape(KT, 128, BS)
        xb[:, :, BS] = blo
        xb[:, :, BS + 1] = bhi
        in_maps.append({
            "xb": xb, "maskT": maskT, "elT": elT, "ehT": ehT,
            "t_row": t_row, "w_bcast": w_b,
        })
    return in_maps, float(-khalf), float(khalf)


def _reference_numpy(x, center, log_width, e_low, e_high, mask, log_kappa, t,
                     head_w, head_b):
    """General fallback, exact reference semantics in fp32 numpy (chunked)."""
    width = np.clip(np.exp(log_width, dtype=_F32), 1e-3, 50.0).astype(_F32)
    t_low = (center - _F32(0.5) * width).astype(_F32)
    t_high = (center + _F32(0.5) * width).astype(_F32)
    kappa = np.clip(np.exp(_F32(log_kappa)), 0.5, 50.0).astype(_F32)

    def sig(v):
        return _F32(0.5) * (np.tanh(_F32(0.5) * v) + _F32(1.0))

    m = sig(mask.astype(_F32))
    el = np.tanh(e_low.astype(_F32))
    eh = np.tanh(e_high.astype(_F32))
    out = np.empty(x.shape[0], dtype=_F32)
    for s in range(0, x.shape[0], 64):
        xc = x[s:s + 64].astype(_F32)
        low = sig(kappa * (t_low[None] - xc[:, None, :]))
        high = sig(kappa * (xc[:, None, :] - t_high[None]))
        evidence = np.sum(
            m[None] * (el[None] * (2 * low - 1) + eh[None] * (2 * high - 1)),
            axis=2, dtype=_F32)
        z = sig(_F32(BETA) * (evidence - t[None].astype(_F32)))
        out[s:s + 64] = z @ head_w.reshape(-1).astype(_F32) + _F32(head_b)
    return out


def kernel_with_stats(trace=False, **inputs):
    x = np.asarray(inputs["x"], dtype=_F32)
    center = np.asarray(inputs["center"], dtype=_F32)
    log_width = np.asarray(inputs["log_width"], dtype=_F32)
    e_low = np.asarray(inputs["e_low"], dtype=_F32)
    e_high = np.asarray(inputs["e_high"], dtype=_F32)
    mask = np.asarray(inputs["mask"], dtype=_F32)
    log_kappa = np.asarray(inputs["log_kappa"], dtype=_F32)
    t = np.asarray(inputs["t"], dtype=_F32)
    head_w = np.asarray(inputs["head_w"], dtype=_F32)
    head_b = np.asarray(inputs["head_b"], dtype=_F32)

    assert x.shape == (B, D) and mask.shape == (R, D)

    # fast-path structural check: thresholds constant across the rule axis
    width = np.clip(np.exp(log_width), 1e-3, 50.0).astype(_F32)
    t_low = (center - _F32(0.5) * width).astype(_F32)
    t_high = (center + _F32(0.5) * width).astype(_F32)
    if not (np.all(t_low == t_low[0:1]) and np.all(t_high == t_high[0:1])):
        out = _reference_numpy(x, center, log_width, e_low, e_high, mask,
                               log_kappa, t, head_w, head_b)
        return out, None

    from concourse.bass_utils import run_bass_kernel_spmd

    kappa = np.clip(np.exp(_F32(log_kappa)), 0.5, 50.0).astype(_F32)
    in_maps, scale_lo, scale_hi = _fast_path_inputs(
        x, mask, e_low, e_high, t_low[0], t_high[0], kappa, t, head_w)

    nc = _build_nc(scale_lo, scale_hi, float(head_b.reshape(-1)[0]))
    res = run_bass_kernel_spmd(nc, in_maps, list(range(N_CORES)), trace=trace)
    out = np.concatenate(
        [res.results[i]["y"].reshape(BS) for i in range(N_CORES)]).astype(_F32)
    return out, res


def kernel(**inputs):
    out, _ = kernel_with_stats(**inputs)
    return out


# revision 18
# speedup vs baseline: 1.4528x; 1.4528x over previous
"""Trainium2 Bass kernel for nn_BiEvidenceNet.

Model (B=1024, R=512, D=256):
    width  = clip(exp(log_width), 1e-3, 50)                  (R,D)
    t_low  = center - width/2 ; t_high = center + width/2    (R,D)
    kappa  = clip(exp(log_kappa), 0.5, 50)                   scalar
    low    = sigmoid(kappa*(t_low - x))   high = sigmoid(kappa*(x - t_high))
    evidence[b,r] = sum_d m*(el*(2*low-1) + eh*(2*high-1))   m=sig(mask), el/eh=tanh(e_*)
    z = sigmoid(6*(evidence - t));  y = z @ head_w.T + head_b

Key identity: 2*sigmoid(u)-1 = tanh(u/2). When t_low / t_high are constant
across the rule axis (true at init: center == 0, log_width == 0 -- verified at
runtime), the (B,R,D) broadcast collapses to two matmuls:
    T_low[b,d]  = tanh(kappa/2*(tau_low[d]  - x[b,d]))
    T_high[b,d] = tanh(kappa/2*(x[b,d] - tau_high[d]))
    evidence    = T_low @ (m*el).T + T_high @ (m*eh).T
Sharding: data-parallel over B, 128 rows/core; (R,D) params replicated.
On-core layout keeps D on partitions (2 k-tiles of 128) so both matmul
operands are naturally transposed; evidence lands as (128 b, 512 r) in PSUM,
-t enters as two rank-1 matmul updates, and the head is a DVE
multiply+reduce over the free (rule) axis followed by a PE transpose so the
output leaves as one contiguous row (a partition-strided 4B-per-partition
store pays ~7us of HWDGE semaphore latency).

Toolchain constraint baked in throughout: this walrus encodes at most ONE
sync wait per instruction, so the instruction graph is arranged so every op
has a single-semaphore dependency (see the `ones`/`wcheck` covering ops).
"""

import numpy as np

B, R, D = 1024, 512, 256
N_CORES = 8
BS = B // N_CORES          # batch rows per core
KT = D // 128              # contraction k-tiles
BETA = 6.0
TRIM_TAIL = True           # skip Tile's sem-clear + second barrier (one-shot NEFF)

_F32 = np.float32


def _single_wait_tile_context(nc, tile):
    """TileContext whose tail carries at most one sync wait per instruction."""
    from concourse.vector_clock import ScopedClock, VectorClock

    class SingleWaitTileContext(tile.TileContext):
        def _drain_and_barrier(self, tick_clock, wait_clock):
            gc = tick_clock.global_clock
            n = len(gc)
            for proc in range(n):
                if gc[proc] <= 0:
                    continue
                vec = VectorClock([gc[i] if i == proc else 0 for i in range(n)])
                inst = self.nc.sync.nop(nofuse=True)
                wait_clock.add_sem_waits(inst.ins, ScopedClock({None: vec}))
            # the NOP chain above already waited out every proc, so the drain
            # itself needs no waits (walrus would reject a multi-wait drain)
            self.nc.sync.drain()
            self.nc.all_engine_barrier()
            assert self.sems is not None
            popped = self.nc._tile_sem_poison_stack.pop()
            assert popped is self._sem_poison
            if not TRIM_TAIL:
                self.nc.clear_and_free_semaphores(
                    list(self.sems.allocated().values()))
                self.nc.all_engine_barrier()

    return SingleWaitTileContext(nc)


def _build_nc(scale_lo: float, scale_hi: float, head_b: float):
    import concourse.bass as bass
    import concourse.mybir as mybir
    from concourse import tile

    f32 = mybir.dt.float32
    f32r = mybir.dt.float32r
    AF = mybir.ActivationFunctionType
    ALU = mybir.AluOpType

    nc = bass.Bass()
    # xb packs the x shard (transposed) with the two per-partition activation
    # bias columns so each T activation depends on exactly one DMA semaphore.
    d_xb = nc.declare_dram_parameter("xb", [KT, 128, BS + 2], f32, isOutput=False)
    d_maskT = nc.declare_dram_parameter("maskT", [KT, 128, R], f32, isOutput=False)
    d_elT = nc.declare_dram_parameter("elT", [KT, 128, R], f32, isOutput=False)
    d_ehT = nc.declare_dram_parameter("ehT", [KT, 128, R], f32, isOutput=False)
    d_t = nc.declare_dram_parameter("t_row", [1, R], f32, isOutput=False)
    # head_w broadcast to 128 partitions, with a 128x128 identity appended
    # (used to transpose y into a single contiguous output row)
    d_wbi = nc.declare_dram_parameter("wbi", [BS, R + BS], f32, isOutput=False)
    d_y = nc.declare_dram_parameter("y", [1, BS], f32, isOutput=True)

    with _single_wait_tile_context(nc, tile) as tc:
        with (
            tc.tile_pool(name="sb", bufs=1) as sb,
            tc.tile_pool(name="ps", bufs=1, space="PSUM") as ps,
        ):
            mkt = sb.tile([128, KT, R], f32, tag="mkt")
            elt = sb.tile([128, KT, R], f32, tag="elt")
            eht = sb.tile([128, KT, R], f32, tag="eht")
            xt = sb.tile([128, KT, BS + 2], f32, tag="xt")
            tr = sb.tile([1, R], f32, tag="tr")
            wbi = sb.tile([BS, R + BS], f32, tag="wbi")

            # big replicated params first so their queues start streaming
            # ASAP (they gate the prep activations on the critical path);
            # one DMA per logical tensor -> one wait per consumer
            nc.sync.dma_start(mkt[:], d_maskT[:].rearrange("k p r -> p k r"))
            nc.sync.dma_start(elt[:], d_elT[:].rearrange("k p r -> p k r"))
            nc.sync.dma_start(eht[:], d_ehT[:].rearrange("k p r -> p k r"))
            nc.sync.dma_start(xt[:], d_xb[:].rearrange("k p b -> p k b"))
            nc.sync.dma_start(tr[:], d_t[:])
            nc.sync.dma_start(wbi[:], d_wbi[:])

            # T tiles, kept transposed: partition = d, free = b
            tlo = sb.tile([128, KT, BS], f32r, tag="tlo")
            thi = sb.tile([128, KT, BS], f32r, tag="thi")
            for k in range(KT):
                nc.scalar.activation(
                    tlo[:, k, :], xt[:, k, 0:BS], AF.Tanh,
                    bias=xt[:, k, BS:BS + 1], scale=scale_lo,
                )
            for k in range(KT):
                nc.scalar.activation(
                    thi[:, k, :], xt[:, k, 0:BS], AF.Tanh,
                    bias=xt[:, k, BS + 1:BS + 2], scale=scale_hi,
                )

            # rule-param prep: A = sigmoid(mask)*tanh(e_low), B2 = ... e_high
            m = sb.tile([128, KT, R], f32, tag="m")
            el = sb.tile([128, KT, R], f32, tag="el")
            eh = sb.tile([128, KT, R], f32, tag="eh")
            nc.scalar.activation(m[:], mkt[:], AF.Sigmoid)
            nc.scalar.activation(el[:], elt[:], AF.Tanh)
            nc.scalar.activation(eh[:], eht[:], AF.Tanh)

            # DVE chain; `ones*` read a_t/b_t so a PE wait on their tick
            # transitively covers the matmul operands (single-wait rule);
            # `wcheck` pulls the wbi DMA tick onto DVE for the head ops
            negth = sb.tile([1, R], f32r, tag="negth")
            nc.vector.tensor_scalar_mul(negth[:], tr[:], -0.5)
            wcheck = sb.tile([1, 1], f32, tag="wcheck")
            nc.vector.tensor_scalar_mul(wcheck[:], wbi[0:1, 0:1], 1.0)
            a_t = sb.tile([128, KT, R], f32r, tag="a_t")
            b_t = sb.tile([128, KT, R], f32r, tag="b_t")
            nc.vector.tensor_mul(a_t[:], m[:], el[:])
            ones = sb.tile([1, BS], f32r, tag="ones")
            nc.vector.tensor_scalar(
                ones[:], a_t[0:1, 0, 0:BS], 0.0, 1.0, ALU.mult, ALU.add)
            nc.vector.tensor_mul(b_t[:], m[:], eh[:])
            ones2 = sb.tile([1, BS], f32r, tag="ones2")
            nc.vector.tensor_scalar(
                ones2[:], b_t[0:1, 0, 0:BS], 0.0, 1.0, ALU.mult, ALU.add)

            # evidence - t, accumulated in one PSUM bank: (128 b, 512 r).
            # float32r runs the PE at full rate for free dim >= 256 (plain
            # fp32 is quarter-rate).  The -t rank-1 term is split in two so
            # each half can carry the DVE wait that covers a_t / b_t.
            ev = ps.tile([128, R], f32, tag="ev")

            # dummy matmul whose only dependency is the wbi DMA: it makes the
            # PE observe that queue's semaphore so the final transpose matmul
            # (which also reads wbi) needs only its DVE wait
            scratch_ps = ps.tile([128, 1], f32, tag="scratch_ps")
            nc.tensor.matmul(scratch_ps[:], wbi[:, R:R + BS], wbi[:, R:R + 1],
                             start=True, stop=True)

            def mm(lhsT, rhs, start, stop):
                nc.tensor.matmul(ev[:], lhsT, rhs, start=start, stop=stop)

            mm(ones[:], negth[:], True, False)
            mm(tlo[:, 0, :], a_t[:, 0, :], False, False)
            mm(tlo[:, 1, :], a_t[:, 1, :], False, False)
            mm(ones2[:], negth[:], False, False)
            mm(thi[:, 0, :], b_t[:, 0, :], False, False)
            mm(thi[:, 1, :], b_t[:, 1, :], False, True)

            z = sb.tile([128, R], f32, tag="z")
            nc.scalar.activation(z[:], ev[:], AF.Sigmoid, scale=BETA)

            # head: y[b] = sum_r z*w + head_b, then transpose the (128,1)
            # column into a (1,128) row on the PE so the output DMA is one
            # contiguous packet instead of 4 bytes per partition
            zw = sb.tile([128, R], f32, tag="zw")
            yt = sb.tile([128, 1], f32, tag="yt")
            nc.vector.tensor_mul(zw[:], z[:], wbi[:, 0:R])
            nc.vector.tensor_reduce(
                yt[:], zw[:], axis=mybir.AxisListType.X, op=ALU.add)
            nc.vector.tensor_scalar_add(yt[:], yt[:], head_b)

            yrow_ps = ps.tile([1, BS], f32, tag="yrow_ps")
            nc.tensor.matmul(yrow_ps[:], yt[:], wbi[:, R:R + BS],
                             start=True, stop=True)
            yrow = sb.tile([1, BS], f32, tag="yrow")
            nc.scalar.activation(yrow[:], yrow_ps[:], AF.Identity)
            nc.sync.dma_start(d_y[:], yrow[:])

    nc.finalize()
    return nc


def _fast_path_inputs(x, mask, e_low, e_high, tau_lo, tau_hi, kappa, t, head_w):
    """Build the per-core input maps (host work = transposes/slicing only)."""
    khalf = _F32(kappa) / _F32(2.0)
    blo = (khalf * tau_lo).astype(_F32).reshape(KT, 128)
    bhi = (-khalf * tau_hi).astype(_F32).reshape(KT, 128)
    maskT = np.ascontiguousarray(mask.T.reshape(KT, 128, R), dtype=_F32)
    elT = np.ascontiguousarray(e_low.T.reshape(KT, 128, R), dtype=_F32)
    ehT = np.ascontiguousarray(e_high.T.reshape(KT, 128, R), dtype=_F32)
    t_row = np.ascontiguousarray(t.reshape(1, R), dtype=_F32)
    wbi = np.empty((BS, R + BS), dtype=_F32)
    wbi[:, :R] = head_w.reshape(1, R)
    wbi[:, R:] = np.eye(BS, dtype=_F32)
    xT = np.ascontiguousarray(x.T, dtype=_F32)  # (D, B)

    in_maps = []
    for i in range(N_CORES):
        xb = np.empty((KT, 128, BS + 2), dtype=_F32)
        xb[:, :, :BS] = xT[:, i * BS:(i + 1) * BS].reshape(KT, 128, BS)
        xb[:, :, BS] = blo
        xb[:, :, BS + 1] = bhi
        in_maps.append({
            "xb": xb, "maskT": maskT, "elT": elT, "ehT": ehT,
            "t_row": t_row, "wbi": wbi,
        })
    return in_maps, float(-khalf), float(khalf)


def _reference_numpy(x, center, log_width, e_low, e_high, mask, log_kappa, t,
                     head_w, head_b):
    """General fallback, exact reference semantics in fp32 numpy (chunked)."""
    width = np.clip(np.exp(log_width, dtype=_F32), 1e-3, 50.0).astype(_F32)
    t_low = (center - _F32(0.5) * width).astype(_F32)
    t_high = (center + _F32(0.5) * width).astype(_F32)
    kappa = np.clip(np.exp(_F32(log_kappa)), 0.5, 50.0).astype(_F32)

    def sig(v):
        return _F32(0.5) * (np.tanh(_F32(0.5) * v) + _F32(1.0))

    m = sig(mask.astype(_F32))
    el = np.tanh(e_low.astype(_F32))
    eh = np.tanh(e_high.astype(_F32))
    out = np.empty(x.shape[0], dtype=_F32)
    for s in range(0, x.shape[0], 64):
        xc = x[s:s + 64].astype(_F32)
        low = sig(kappa * (t_low[None] - xc[:, None, :]))
        high = sig(kappa * (xc[:, None, :] - t_high[None]))
        evidence = np.sum(
            m[None] * (el[None] * (2 * low - 1) + eh[None] * (2 * high - 1)),
            axis=2, dtype=_F32)
        z = sig(_F32(BETA) * (evidence - t[None].astype(_F32)))
        out[s:s + 64] = z @ head_w.reshape(-1).astype(_F32) + _F32(head_b)
    return out


def kernel_with_stats(trace=False, **inputs):
    x = np.asarray(inputs["x"], dtype=_F32)
    center = np.asarray(inputs["center"], dtype=_F32)
    log_width = np.asarray(inputs["log_width"], dtype=_F32)
    e_low = np.asarray(inputs["e_low"], dtype=_F32)
    e_high = np.asarray(inputs["e_high"], dtype=_F32)
    mask = np.asarray(inputs["mask"], dtype=_F32)
    log_kappa = np.asarray(inputs["log_kappa"], dtype=_F32)
    t = np.asarray(inputs["t"], dtype=_F32)
    head_w = np.asarray(inputs["head_w"], dtype=_F32)
    head_b = np.asarray(inputs["head_b"], dtype=_F32)

    assert x.shape == (B, D) and mask.shape == (R, D)

    # fast-path structural check: thresholds constant across the rule axis
    width = np.clip(np.exp(log_width), 1e-3, 50.0).astype(_F32)
    t_low = (center - _F32(0.5) * width).astype(_F32)
    t_high = (center + _F32(0.5) * width).astype(_F32)
    if not (np.all(t_low == t_low[0:1]) and np.all(t_high == t_high[0:1])):
        out = _reference_numpy(x, center, log_width, e_low, e_high, mask,
                               log_kappa, t, head_w, head_b)
        return out, None

    from concourse.bass_utils import run_bass_kernel_spmd

    kappa = np.clip(np.exp(_F32(log_kappa)), 0.5, 50.0).astype(_F32)
    in_maps, scale_lo, scale_hi = _fast_path_inputs(
        x, mask, e_low, e_high, t_low[0], t_high[0], kappa, t, head_w)

    nc = _build_nc(scale_lo, scale_hi, float(head_b.reshape(-1)[0]))
    res = run_bass_kernel_spmd(nc, in_maps, list(range(N_CORES)), trace=trace)
    out = np.concatenate(
        [res.results[i]["y"].reshape(BS) for i in range(N_CORES)]).astype(_F32)
    return out, res


def kernel(**inputs):
    out, _ = kernel_with_stats(**inputs)
    return out
